# revision 2
# baseline (speedup 1.0000x reference)
"""CrossModalAttention Trainium2 kernel (v3).

Full inputs in, full outputs out; internally sharded data-parallel over the
batch dim across 8 NeuronCores (4 batch items per core).

Per batch item (C=256, H=W=64, AS=8, T=64):
  - Host pre-casts F_d -> fp16 and F_rgb -> (1-alpha)*F_rgb in fp16 (halves
    DMA read bytes; the (1-alpha) blend scale rides the cast for free and is
    unfolded via the Q weights).
  - F_d is loaded via SWDGE accumulate-DMA chains (4 transfers/item, CCE
    inline adders do the first h-pool level for free); the rest of the 8x8
    avgpool runs as fp16 pairwise-add trees on DVE.
  - F_rgb is loaded plain (full tile needed for the blend); its avgpool is a
    6-level DVE tree. The 1/64 mean (and 1/(1-alpha)) folded into weights.
  - Attention core batched per item PAIR (rows (b,t), cols (b,s)) with a
    -30000 additive mask on cross-item blocks so one softmax handles both.
  - Q = Wq@R+bq, K = Wk@D+bk as [o, (b,t)] (PE fp16 + ACT bias, fp16 out)
  - VfT = D^T @ Wv^T + ones^T@bv as [(b,s), o] (PE fp16)
  - A = Qf^T Kf + mask [128,128] fp32 PSUM; softmax rows (DVE + ACT exp)
  - AsmT via PE transpose; FattT = AsmT_cols @ VfT per item [t, c]
  - upsample: psum = FattT_chunk^T @ (alpha*kron(U^T,U^T))_block per
    512-wide block (PE fp16) and the blend (1-a)*F_rgb accumulated into the
    same PSUM via an identity-weight matmul; ACT/DVE copy psum -> fp16 out
    tile; one DMA store per batch item (ACT HWDGE queue; inputs ride the
    sync queue so stores never block loads); host upcasts to fp32.
"""

import numpy as np
from contextlib import ExitStack

import concourse.bacc as bacc
import concourse.mybir as mybir
import concourse.tile as tile
from concourse.bass_utils import run_bass_kernel_spmd

B, C, H, W = 32, 256, 64, 64
AS = 8
T = AS * AS          # 64 pooled pixels
HW = H * W           # 4096
NCORES = 8
BPC = B // NCORES    # 4 batch items per core
NCHUNK = C // 128    # 2 channel chunks

F32 = mybir.dt.float32
F16 = mybir.dt.float16
NPF16 = np.float16
ADD = mybir.AluOpType.add


def _bilinear_up_matrix(n_out: int, n_in: int) -> np.ndarray:
    """U[i, p]: weight of coarse pixel p for fine pixel i; half-pixel centers
    with edge clamping (identical to jax.image.resize bilinear upsample)."""
    U = np.zeros((n_out, n_in), np.float64)
    scale = n_in / n_out
    for i in range(n_out):
        src = (i + 0.5) * scale - 0.5
        p0 = int(np.floor(src))
        f = src - p0
        for p, wgt in ((p0, 1.0 - f), (p0 + 1, f)):
            pc = min(max(p, 0), n_in - 1)
            U[i, pc] += wgt
    return U


_CACHE = {}


def _build_program(blend: bool):
    nc = bacc.Bacc("TRN2", target_bir_lowering=False, debug=False,
                   num_devices=NCORES)

    frgb = nc.dram_tensor("frgb", [BPC, NCHUNK, 128, HW], F16,
                          kind="ExternalInput").ap()
    # fdp[b, j, ci, c, vb, hp*64+w] = F_d[b, ci*128+c, (hp*8 + vb*4 + j)*64+w]
    fdp = nc.dram_tensor("fdp", [BPC, 4, NCHUNK, 128, 2, 512], F16,
                         kind="ExternalInput").ap()
    wqt = nc.dram_tensor("wqt", [NCHUNK, 128, C], F16, kind="ExternalInput").ap()
    wkt = nc.dram_tensor("wkt", [NCHUNK, 128, C], F16, kind="ExternalInput").ap()
    wvt = nc.dram_tensor("wvt", [NCHUNK, 128, C], F16, kind="ExternalInput").ap()
    bq2 = nc.dram_tensor("bq2", [128, NCHUNK], F32, kind="ExternalInput").ap()
    bk2 = nc.dram_tensor("bk2", [128, NCHUNK], F32, kind="ExternalInput").ap()
    bvr = nc.dram_tensor("bvr", [1, C], F16, kind="ExternalInput").ap()
    u2a = nc.dram_tensor("u2a", [T, HW], F16, kind="ExternalInput").ap()
    id128 = nc.dram_tensor("id128", [128, 128], F16, kind="ExternalInput").ap()
    ones128 = nc.dram_tensor("ones128", [1, 128], F16, kind="ExternalInput").ap()
    maskl = nc.dram_tensor("maskl", [2, 128], F16, kind="ExternalInput").ap()
    maskr = nc.dram_tensor("maskr", [2, 128], F16, kind="ExternalInput").ap()
    out = nc.dram_tensor("out", [BPC, NCHUNK, 128, HW], F16,
                         kind="ExternalOutput").ap()

    with tile.TileContext(nc) as tc, ExitStack() as ctx:
        consts = ctx.enter_context(tc.tile_pool(name="consts", bufs=1))
        fr_pool = ctx.enter_context(tc.tile_pool(name="fr", bufs=4))
        fd_pool = ctx.enter_context(tc.tile_pool(name="fdp", bufs=2))
        out_pool = ctx.enter_context(tc.tile_pool(name="outp", bufs=2))
        scr_pool = ctx.enter_context(tc.tile_pool(name="scr", bufs=2))
        small = ctx.enter_context(tc.tile_pool(name="small", bufs=2))
        ps_small = ctx.enter_context(
            tc.tile_pool(name="pss", bufs=2, space="PSUM"))
        ps_out = ctx.enter_context(
            tc.tile_pool(name="pso", bufs=3, space="PSUM"))

        # ---- input loads first (sync/gpsimd queues), consts on ACT queue ----
        fr_ts = []
        for b in range(BPC):
            fr_t = fr_pool.tile([128, NCHUNK * HW], F16, tag="fr")
            nc.sync.dma_start(fr_t.rearrange("p (a b) -> p a b", a=NCHUNK),
                              frgb[b].transpose([1, 0, 2]))
            fr_ts.append(fr_t)

        fd_xy = []
        for b in range(BPC):
            # accumulate-DMA chains: X = fd(v=0)+fd(v=1), Y = fd(v=2)+fd(v=3)
            # per vb half (layout [p, (ci, vb, hp*64+w)])
            x_t = fd_pool.tile([128, NCHUNK * 1024], F16, tag="fdx")
            y_t = fd_pool.tile([128, NCHUNK * 1024], F16, tag="fdy")
            for j, dst in ((0, x_t), (1, x_t), (2, y_t), (3, y_t)):
                nc.gpsimd.dma_start(
                    dst.rearrange("p (a v b) -> p a v b", a=NCHUNK, v=2),
                    fdp[b, j].transpose([1, 0, 2, 3]),
                    accum_op=(ADD if j % 2 == 1 else mybir.AluOpType.bypass))
            fd_xy.append((x_t, y_t))

        # ---- constants into SBUF (ACT HWDGE queue) ----
        wqt_s = consts.tile([128, NCHUNK * C], F16)   # [c, (ci, o)]
        nc.scalar.dma_start(wqt_s.rearrange("p (a b) -> p a b", a=NCHUNK),
                            wqt.transpose([1, 0, 2]))
        wkt_s = consts.tile([128, NCHUNK * C], F16)
        nc.scalar.dma_start(wkt_s.rearrange("p (a b) -> p a b", a=NCHUNK),
                            wkt.transpose([1, 0, 2]))
        wvt_s = consts.tile([128, NCHUNK * C], F16)
        nc.scalar.dma_start(wvt_s.rearrange("p (a b) -> p a b", a=NCHUNK),
                            wvt.transpose([1, 0, 2]))
        bq_s = consts.tile([128, NCHUNK], F32)
        nc.scalar.dma_start(bq_s[:], bq2[:])
        bk_s = consts.tile([128, NCHUNK], F32)
        nc.scalar.dma_start(bk_s[:], bk2[:])
        bvr_s = consts.tile([1, C], F16)
        nc.scalar.dma_start(bvr_s[:], bvr[:])
        u2a_s = consts.tile([T, HW], F16)
        nc.scalar.dma_start(u2a_s[:], u2a[:])
        id128_s = consts.tile([128, 128], F16)
        nc.scalar.dma_start(id128_s[:], id128[:])
        ones_s = consts.tile([1, 128], F16)
        nc.scalar.dma_start(ones_s[:], ones128[:])
        maskl_s = consts.tile([2, 128], F16)
        nc.scalar.dma_start(maskl_s[:], maskl[:])
        maskr_s = consts.tile([2, 128], F16)
        nc.scalar.dma_start(maskr_s[:], maskr[:])

        eng = nc.vector

        for pr in range(BPC // 2):          # item pairs
            rs_t = small.tile([128, NCHUNK * 128], F16, tag="rs")  # [c,(ci,q,t)]
            ds_t = small.tile([128, NCHUNK * 128], F16, tag="ds")

            for q in range(2):
                b = 2 * pr + q
                fr_t = fr_ts[b]

                # ---- F_rgb avgpool: 6-level DVE tree ----
                # fr layout [p, (ci, hp, v, w)]; (ci,hp) fuses: stride 512.
                frv = fr_t.rearrange("p (a v w) -> p a v w", a=2 * AS * AS // 8,
                                     v=AS)   # [p, 16, 8, 64]
                w1 = scr_pool.tile([128, 4096], F16, tag="w1")
                w1v = w1.rearrange("p (a v w) -> p a v w", a=16, v=4)
                eng.tensor_add(w1v, frv[:, :, 0:4, :], frv[:, :, 4:8, :])
                w2 = scr_pool.tile([128, 2048], F16, tag="w2")
                w2v = w2.rearrange("p (a v w) -> p a v w", a=16, v=2)
                eng.tensor_add(w2v, w1v[:, :, 0:2, :], w1v[:, :, 2:4, :])
                r1 = scr_pool.tile([128, 1024], F16, tag="r1")
                r1v = r1.rearrange("p (a v w) -> p a v w", a=16, v=1)
                eng.tensor_add(r1v, w2v[:, :, 0:1, :], w2v[:, :, 1:2, :])
                # w-levels on [p, 16, wq, u]
                r1w = r1.rearrange("p (a b u) -> p a b u", a=16, b=AS)
                t1 = scr_pool.tile([128, 512], F16, tag="t1")
                t1v = t1.rearrange("p (a b u) -> p a b u", a=16, b=AS)
                eng.tensor_add(t1v, r1w[:, :, :, 0:4], r1w[:, :, :, 4:8])
                t2 = scr_pool.tile([128, 256], F16, tag="t2")
                t2v = t2.rearrange("p (a b u) -> p a b u", a=16, b=AS)
                eng.tensor_add(t2v, t1v[:, :, :, 0:2], t1v[:, :, :, 2:4])
                # final level strided into rs[:, ci, q, t]
                dst = rs_t.rearrange("p (ci q hp wq) -> p ci q hp wq",
                                     ci=NCHUNK, q=2, hp=AS)[:, :, q]
                t2s = t2.rearrange("p (ci hp wq u) -> p ci hp wq u",
                                   ci=NCHUNK, hp=AS, wq=AS)
                eng.tensor_add(dst, t2s[:, :, :, :, 0], t2s[:, :, :, :, 1])

                # ---- F_d avgpool tail (DMA already did the first level) ----
                x_t, y_t = fd_xy[b]
                s1 = scr_pool.tile([128, 2048], F16, tag="s1")
                eng.tensor_add(s1[:], x_t[:], y_t[:])
                g = scr_pool.tile([128, 1024], F16, tag="g")
                s1v = s1.rearrange("p (a v b) -> p a v b", a=NCHUNK, v=2)
                gv = g.rearrange("p (a v b) -> p a v b", a=NCHUNK, v=1)
                eng.tensor_add(gv, s1v[:, :, 0:1, :], s1v[:, :, 1:2, :])
                gw = g.rearrange("p (a b u) -> p a b u", a=16, b=AS)
                d1 = scr_pool.tile([128, 512], F16, tag="d1")
                d1v = d1.rearrange("p (a b u) -> p a b u", a=16, b=AS)
                eng.tensor_add(d1v, gw[:, :, :, 0:4], gw[:, :, :, 4:8])
                d2 = scr_pool.tile([128, 256], F16, tag="d2")
                d2v = d2.rearrange("p (a b u) -> p a b u", a=16, b=AS)
                eng.tensor_add(d2v, d1v[:, :, :, 0:2], d1v[:, :, :, 2:4])
                dstd = ds_t.rearrange("p (ci q hp wq) -> p ci q hp wq",
                                      ci=NCHUNK, q=2, hp=AS)[:, :, q]
                d2s = d2.rearrange("p (ci hp wq u) -> p ci hp wq u",
                                   ci=NCHUNK, hp=AS, wq=AS)
                eng.tensor_add(dstd, d2s[:, :, :, :, 0], d2s[:, :, :, :, 1])

            # ---- Q, K: [o, (q,t)] with per-partition bias (fp16 out) ----
            qf_t = small.tile([128, NCHUNK * 128], F16, tag="qf")  # [(o),(oj,qt)]
            kf_t = small.tile([128, NCHUNK * 128], F16, tag="kf")
            for w_s, b_s, sums, dst in ((wqt_s, bq_s, rs_t, qf_t),
                                        (wkt_s, bk_s, ds_t, kf_t)):
                for oj in range(NCHUNK):
                    psq = ps_small.tile([128, 128], F32, tag="pss")
                    for ci in range(NCHUNK):
                        nc.tensor.matmul(
                            psq[:],
                            w_s[:, ci * C + oj * 128: ci * C + (oj + 1) * 128],
                            sums[:, ci * 128:(ci + 1) * 128],
                            start=(ci == 0), stop=(ci == NCHUNK - 1))
                    nc.scalar.activation(
                        dst[:, oj * 128:(oj + 1) * 128], psq[:],
                        mybir.ActivationFunctionType.Identity,
                        bias=b_s[:, oj:oj + 1], scale=1.0)

            # ---- VfT = D^T Wv^T + ones^T bv : [(q,s), o] ----
            psv = ps_small.tile([128, C], F32, tag="pss")
            for ci in range(NCHUNK):
                nc.tensor.matmul(psv[:],
                                 ds_t[:, ci * 128:(ci + 1) * 128],
                                 wvt_s[:, ci * C:(ci + 1) * C],
                                 start=(ci == 0), stop=False)
            nc.tensor.matmul(psv[:], ones_s[:, 0:128], bvr_s[:],
                             start=False, stop=True)
            vft = small.tile([128, C], F16, tag="vft")
            nc.scalar.copy(vft[:], psv[:])

            # ---- A = Qf^T Kf + crossmask : [(q,t), (q,s)] ----
            psa = ps_small.tile([128, 128], F32, tag="pss")
            for oj in range(NCHUNK):
                nc.tensor.matmul(psa[:],
                                 qf_t[:, oj * 128:(oj + 1) * 128],
                                 kf_t[:, oj * 128:(oj + 1) * 128],
                                 start=(oj == 0), stop=False)
            nc.tensor.matmul(psa[:], maskl_s[:], maskr_s[:],
                             start=False, stop=True)

            # ---- softmax over free dim ----
            negmax = small.tile([128, 1], F32, tag="negmax")
            nc.vector.tensor_reduce(negmax[:], psa[:],
                                    axis=mybir.AxisListType.X,
                                    op=mybir.AluOpType.max, negate=True)
            e_t = small.tile([128, 128], F32, tag="e")
            nc.scalar.activation(e_t[:], psa[:],
                                 mybir.ActivationFunctionType.Exp,
                                 bias=negmax[:, 0:1], scale=1.0)
            s1r = small.tile([128, 1], F32, tag="s1r")
            nc.vector.reduce_sum(s1r[:], e_t[:], axis=mybir.AxisListType.X)
            rcp = small.tile([128, 1], F32, tag="rcp")
            nc.vector.reciprocal(rcp[:], s1r[:])
            asm = small.tile([128, 128], F16, tag="asm")
            nc.vector.tensor_scalar_mul(asm[:], e_t[:], rcp[:, 0:1])

            # ---- AsmT via PE transpose ----
            psat = ps_small.tile([128, 128], F16, tag="pss")
            nc.tensor.transpose(psat[:], asm[:], id128_s[:])
            asmt = small.tile([128, 128], F16, tag="asmt")
            nc.vector.tensor_copy(asmt[:], psat[:])

            # ---- FattT per item: [t, c] = asmt_cols @ VfT ----
            ft_q = []
            for q in range(2):
                psf = ps_small.tile([T, C], F32, tag="pss")
                nc.tensor.matmul(psf[:], asmt[:, q * T:(q + 1) * T], vft[:],
                                 start=True, stop=True)
                ftq = small.tile([T, C], F16, tag=f"ft{q}")
                nc.scalar.copy(ftq[:], psf[:])
                ft_q.append(ftq)

            # ---- upsample (PE) + blend accum (PE) -> copies -> store ----
            for q in range(2):
                b = 2 * pr + q
                fr_t = fr_ts[b]
                ftq = ft_q[q]
                out_t = out_pool.tile([128, NCHUNK * HW], F16, tag="ot")
                blk = 0
                for ci in range(NCHUNK):
                    for nb in range(HW // 1024):
                        off = ci * HW + nb * 1024
                        pso = ps_out.tile([128, 1024], F32, tag="pso")
                        for hb in range(2):
                            ps_half = pso[:, hb * 512:(hb + 1) * 512]
                            ucol = nb * 1024 + hb * 512
                            nc.tensor.matmul(
                                ps_half,
                                ftq[:, ci * 128:(ci + 1) * 128],
                                u2a_s[:, ucol:ucol + 512],
                                start=True, stop=not blend)
                            if blend:
                                nc.tensor.matmul(
                                    ps_half, id128_s[:],
                                    fr_t[:, off + hb * 512:off + (hb + 1) * 512],
                                    start=False, stop=True)
                        # copy psum -> out tile; one block per item on DVE
                        if blk == 3:
                            nc.vector.tensor_copy(out_t[:, off:off + 1024],
                                                  pso[:])
                        else:
                            nc.scalar.copy(out_t[:, off:off + 1024], pso[:])
                        blk += 1
                nc.scalar.dma_start(out[b].transpose([1, 0, 2]),
                                    out_t.rearrange("p (a b) -> p a b",
                                                    a=NCHUNK))

    nc.compile()
    return nc


def _prepare_in_maps(F_rgb, F_d, Wq, bq, Wk, bk, Wv, bv, alpha):
    if "U" not in _CACHE:
        _CACHE["U"] = _bilinear_up_matrix(H, AS)
    U = _CACHE["U"]

    a = float(np.asarray(alpha))
    blend = abs(1.0 - a) > 1e-7
    rscale = (1.0 - a) if blend else 1.0

    F_rgb = (np.asarray(F_rgb, np.float32) * np.float32(rscale)).astype(NPF16)
    F_d = np.asarray(F_d, np.float32).astype(NPF16)

    frgb_sh = F_rgb.reshape(NCORES, BPC, NCHUNK, 128, HW)
    # fdp[b, j, ci, c, vb, hp*64+w] = F_d[b, ci*128+c, (hp*8+vb*4+j)*64+w]
    fd7 = F_d.reshape(B, NCHUNK, 128, AS, 2, 4, W)   # b ci c hp vb j w
    fdp = np.ascontiguousarray(fd7.transpose(0, 5, 1, 2, 4, 3, 6)).reshape(
        NCORES, BPC, 4, NCHUNK, 128, 2, 512)

    def wfold(Wx, extra=1.0):
        # [c, o] chunks of (Wx / 64 / extra)^T
        return np.ascontiguousarray(
            (np.asarray(Wx, np.float64).T / (AS * AS * extra))
            .reshape(NCHUNK, 128, C)).astype(NPF16)

    wqt = wfold(Wq, extra=rscale)   # R sums are pre-scaled by rscale
    wkt = wfold(Wk)
    wvt = wfold(Wv)
    bq2 = np.ascontiguousarray(np.asarray(bq, np.float32).reshape(NCHUNK, 128).T)
    bk2 = np.ascontiguousarray(np.asarray(bk, np.float32).reshape(NCHUNK, 128).T)
    bvr = np.asarray(bv, np.float32).reshape(1, C).astype(NPF16)
    u2a = (a * np.kron(U.T, U.T)).astype(NPF16)
    id128 = np.eye(128, dtype=np.float32).astype(NPF16)
    ones128 = np.ones((1, 128), NPF16)
    maskl = np.zeros((2, 128), NPF16)
    maskl[0, 0:64] = 1.0
    maskl[1, 64:128] = 1.0
    maskr = np.zeros((2, 128), NPF16)
    maskr[0, 64:128] = -30000.0
    maskr[1, 0:64] = -30000.0

    in_maps = []
    for i in range(NCORES):
        in_maps.append({
            "frgb": np.ascontiguousarray(frgb_sh[i]),
            "fdp": np.ascontiguousarray(fdp[i]),
            "wqt": wqt, "wkt": wkt, "wvt": wvt,
            "bq2": bq2, "bk2": bk2, "bvr": bvr,
            "u2a": u2a, "id128": id128, "ones128": ones128,
            "maskl": maskl, "maskr": maskr,
        })
    return in_maps, blend


def _execute(in_maps, blend=True, **kwargs):
    key = f"nc_{blend}"
    if key not in _CACHE:
        _CACHE[key] = _build_program(blend)
    res = run_bass_kernel_spmd(_CACHE[key], in_maps, list(range(NCORES)),
                               **kwargs)
    parts = [res.results[i]["out"].astype(np.float32).reshape(BPC, C, H, W)
             for i in range(NCORES)]
    return np.concatenate(parts, axis=0), res


def kernel(F_rgb, F_d, Wq, bq, Wk, bk, Wv, bv, alpha):
    in_maps, blend = _prepare_in_maps(F_rgb, F_d, Wq, bq, Wk, bk, Wv, bv,
                                      alpha)
    out, _ = _execute(in_maps, blend=blend)
    return out
